# revision 4
# baseline (speedup 1.0000x reference)
"""BoundaryLoss kernel for 8 Trainium2 NeuronCores.

Computes mean |pred_dist - target_dist| where *_dist are sums of per-class
exact Euclidean distance transforms of the argmax(pred) / target masks.

Sharding: 8 cores = 4 images x 2 H-halves. Each core computes both masks'
3 per-class EDTs for its half (with +-RK halo rows) and reduces to a
[128,1] partial |diff| sum; the host sums 8 partials and divides.

EDT algorithm per (mask, class, image):
  pass 1 (along W): exact nearest-set-pixel row distances via two
    min-plus scans  state = min(state+1, f)  (forward + backward).
  pass 2 (along H): d^2(x) = min_k (dr[x+k]^2 + k^2) windowed to |k| <= R.
    One fused scalar_tensor_tensor per offset k.

Fast path (the common, dense-mask case): host computes argmax class-id
masks (int8) and two cheap coverage checks that make a fixed window
R=64 and an int16 pipeline provably exact:
  - every 64-wide row segment of every class mask contains a set pixel
    (=> all row distances <= 63, no capping),
  - every 16x16 block contains a set pixel (=> true EDT <= 21.3, so the
    nearest set pixel is within 21 rows; window 64 is exact; every class
    present in every image so no presence flags needed).
Inputs shipped per core are just two [256,256] int8 masks (~1MB total
across cores instead of ~10MB of f32 logits), and the jit dispatch
closure is built once and cached (the generic run_bass_kernel_spmd
rebuilds + retraces a fresh jit every call, ~100ms/call overhead).

If the checks fail (sparse/absent classes), falls back to the general
exact path (data-derived R, on-device argmax, presence flags) via
run_bass_kernel_spmd.
"""

import numpy as np

import concourse.bass as bass
import concourse.bacc as bacc
import concourse.mybir as mybir
from concourse.tile import TileContext
from concourse.bass_utils import run_bass_kernel_spmd

B, C, H, W = 4, 4, 256, 256
N_CORES = 8
LARGEF = 1.0e6  # pseudo-infinity seed for pass-1 scans (pre-square space)
INF = 1 << 20

RK = 64              # fast-path fixed pass-2 window radius
ROWS = 128 + 2 * RK  # rows per core incl. halo (= 256)

F32 = mybir.dt.float32
I32 = mybir.dt.int32
I16 = mybir.dt.int16
I8 = mybir.dt.int8
Alu = mybir.AluOpType
Act = mybir.ActivationFunctionType


# ================================================================ fast path

def _build_fast():
    """Fixed-R (=RK) int16 EDT kernel taking int8 class-id masks."""
    capv = 127.0
    padv = 30000
    rows_pad = ROWS

    nc = bacc.Bacc(None, target_bir_lowering=False)
    maskP = nc.dram_tensor("maskP", [ROWS, W], I8, kind="ExternalInput")
    maskT = nc.dram_tensor("maskT", [ROWS, W], I8, kind="ExternalInput")
    out = nc.dram_tensor("out", [128, 1], F32, kind="ExternalOutput")

    with TileContext(nc) as tc:
        with (
            tc.tile_pool(name="const", bufs=1) as constp,
            tc.tile_pool(name="io", bufs=2) as iop,
            tc.tile_pool(name="p1", bufs=2) as p1p,
            tc.tile_pool(name="h2", bufs=1) as h2p,
            tc.tile_pool(name="fin", bufs=1) as finp,
        ):
            ones = constp.tile([128, W], F32)
            nc.vector.memset(ones[:], 1.0)

            # per-W-chunk transposed row-distance maps, 6 slabs =
            # (pred c1..c3, targ c1..c3). h2A = squared distances; h2B =
            # h2A shifted one element left (keeps odd window offsets on
            # the 2x_1P int16 DVE mode).
            h2d = [h2p.tile([128, 6, rows_pad], I16, name=f"h2d{w}") for w in range(2)]
            h2A = [h2p.tile([128, 6, rows_pad], I16, name=f"h2A{w}") for w in range(2)]
            h2B = [h2p.tile([128, 6, rows_pad], I16, name=f"h2B{w}") for w in range(2)]
            accs = [h2p.tile([128, 6, 128], I16, name=f"acc{w}") for w in range(2)]
            for wc in range(2):
                nc.vector.memset(h2B[wc][:], padv)
                nc.vector.memset(accs[wc][:], padv)

            # ---------------- pass 1 + transpose, per row-chunk
            for cs in (0, 128):
                mpt = iop.tile([128, W], I8, name="mpt")
                nc.gpsimd.dma_start(mpt[:], maskP[cs : cs + 128])
                mtt = iop.tile([128, W], I8, name="mtt")
                nc.gpsimd.dma_start(mtt[:], maskT[cs : cs + 128])
                mpf = p1p.tile([128, W], F32, name="mpf")
                nc.scalar.activation(mpf[:], mpt[:], Act.Copy)
                mtf = p1p.tile([128, W], F32, name="mtf")
                nc.scalar.activation(mtf[:], mtt[:], Act.Copy)

                for slab in range(6):
                    mi, c = divmod(slab, 3)
                    c += 1
                    srcf = mtf if mi == 1 else mpf
                    f = p1p.tile([128, W], F32, name="fseed")
                    nc.vector.tensor_scalar(
                        f[:], srcf[:], float(c), LARGEF,
                        op0=Alu.not_equal, op1=Alu.mult)
                    a = p1p.tile([128, W], F32, name="a")
                    nc.vector.tensor_tensor_scan(
                        a[:], ones[:], f[:], LARGEF,
                        op0=Alu.add, op1=Alu.min)
                    dd = p1p.tile([128, W], F32, name="dd")
                    nc.vector.tensor_tensor_scan(
                        dd[:, ::-1], ones[:], a[:, ::-1], LARGEF,
                        op0=Alu.add, op1=Alu.min)
                    nc.vector.tensor_scalar_min(dd[:], dd[:], capv)
                    ddi = p1p.tile([128, W], I16, name="ddi")
                    nc.gpsimd.tensor_copy(ddi[:], dd[:])

                    for wc in range(2):
                        nc.sync.dma_start_transpose(
                            h2d[wc][:, slab, cs : cs + 128],
                            ddi[:, wc * 128 : (wc + 1) * 128])

            # squares: h2A = h2d^2, h2B = shifted h2A
            for wc in range(2):
                nc.scalar.activation(h2A[wc][:], h2d[wc][:], Act.Square)
                nc.scalar.activation(
                    h2B[wc][:, :, 0 : rows_pad - 1],
                    h2d[wc][:, :, 1:rows_pad], Act.Square)

            # ---------------- pass 2: windowed parabola min-plus along H
            ks = [0]
            for k in range(1, RK + 1):
                ks += [k, -k]
            for k in ks:
                base = RK + k
                kk = k * k
                for wc in range(2):
                    if base % 2 == 1:
                        src, b0 = h2B[wc], base - 1
                    else:
                        src, b0 = h2A[wc], base
                    nc.vector.scalar_tensor_tensor(
                        accs[wc][:], src[:, :, b0 : b0 + 128],
                        int(kk), accs[wc][:],
                        op0=Alu.add, op1=Alu.min)

            # ---------------- sqrt, class sums, |pred-targ|, reduce
            prt = finp.tile([128, 2], F32)
            for wc in range(2):
                sq = finp.tile([128, 6, 128], F32, name="sq")
                for slab in range(6):
                    nc.scalar.activation(
                        sq[:, slab], accs[wc][:, slab], Act.Sqrt)
                sp = finp.tile([128, 128], F32, name="sp")
                st = finp.tile([128, 128], F32, name="st")
                nc.vector.tensor_add(sp[:], sq[:, 0], sq[:, 1])
                nc.vector.tensor_add(sp[:], sp[:], sq[:, 2])
                nc.vector.tensor_add(st[:], sq[:, 3], sq[:, 4])
                nc.vector.tensor_add(st[:], st[:], sq[:, 5])
                nc.vector.tensor_sub(sp[:], sp[:], st[:])
                nc.vector.tensor_reduce(
                    prt[:, wc : wc + 1], sp[:], axis=mybir.AxisListType.X,
                    op=Alu.add, apply_absolute_value=True)
            total = finp.tile([128, 1], F32)
            nc.vector.tensor_add(total[:], prt[:, 0:1], prt[:, 1:2])
            nc.gpsimd.dma_start(out[:], total[:])

    nc.finalize()
    return nc


def _build_runner(nc, n_cores):
    """Build the PJRT dispatch closure ONCE (same lowering path as
    bass_utils.run_bass_kernel_spmd -> bass2jax.run_bass_via_pjrt, with
    the jit hoisted out of the per-call path)."""
    import jax
    from jax.sharding import Mesh, PartitionSpec
    from jax.experimental.shard_map import shard_map
    from concourse import bass2jax as b2j

    b2j.install_neuronx_cc_hook()
    assert nc.dbg_addr is None
    partition_name = (
        nc.partition_id_tensor.name if nc.partition_id_tensor else None
    )

    in_names, out_names, out_avals = [], [], []
    for alloc in nc.m.functions[0].allocations:
        if not isinstance(alloc, mybir.MemoryLocationSet):
            continue
        name = alloc.memorylocations[0].name
        if alloc.kind == "ExternalInput":
            if name != partition_name:
                in_names.append(name)
        elif alloc.kind == "ExternalOutput":
            out_names.append(name)
            shape = tuple(alloc.tensor_shape)
            dtype = mybir.dt.np(alloc.dtype)
            out_avals.append(jax.core.ShapedArray(shape, dtype))
    n_params = len(in_names)
    n_outs = len(out_avals)
    in_names_all = list(in_names) + list(out_names)
    if partition_name is not None:
        in_names_all.append(partition_name)
    in_names_all = tuple(in_names_all)
    donate = tuple(range(n_params, n_params + n_outs))

    def _body(*args):
        operands = list(args)
        if partition_name is not None:
            operands.append(b2j.partition_id_tensor())
        outs = b2j._bass_exec_p.bind(
            *operands,
            out_avals=tuple(out_avals),
            in_names=in_names_all,
            out_names=tuple(out_names),
            lowering_input_output_aliases=(),
            sim_require_finite=True,
            sim_require_nnan=True,
            nc=nc,
        )
        return tuple(outs)

    devices = jax.devices()[:n_cores]
    mesh = Mesh(np.asarray(devices), ("core",))
    sharded = jax.jit(
        shard_map(
            _body, mesh=mesh,
            in_specs=(PartitionSpec("core"),) * (n_params + n_outs),
            out_specs=(PartitionSpec("core"),) * n_outs,
            check_rep=False,
        ),
        donate_argnums=donate, keep_unused=True,
    )
    zero_shapes = [
        ((n_cores * a.shape[0], *a.shape[1:]), a.dtype) for a in out_avals
    ]

    def run(global_inputs):
        """global_inputs: list of [n_cores*dim0, ...] arrays in in_names
        order. Returns list of global output arrays (host numpy)."""
        zeros = [np.zeros(s, d) for s, d in zero_shapes]
        out_arrs = sharded(*global_inputs, *zeros)
        return [np.asarray(o) for o in out_arrs]

    return run


_FAST = {}


def _fast_call(pred, target):
    """Returns the loss, or None if this input needs the general path."""
    p0, p1 = pred[:, 0], pred[:, 1]
    p2, p3 = pred[:, 2], pred[:, 3]
    m01 = np.maximum(p0, p1)
    m23 = np.maximum(p2, p3)
    pm = np.where(
        m23 > m01,
        np.where(p3 > p2, np.int8(3), np.int8(2)),
        np.where(p1 > p0, np.int8(1), np.int8(0)),
    )
    tg = target.astype(np.int8)

    bs = np.stack([pm == 1, pm == 2, pm == 3, tg == 1, tg == 2, tg == 3])
    # every 16x16 block of every (slab, image) has a set pixel
    if not bs.reshape(6, B, 16, 16, 16, 16).any(axis=(3, 5)).all():
        return None
    # every 64-wide row segment has a set pixel
    if not bs.reshape(6, B, H, 4, 64).any(axis=4).all():
        return None

    if "runner" not in _FAST:
        _FAST["runner"] = _build_runner(_build_fast(), N_CORES)
        _FAST["padP"] = np.zeros((B, H + 2 * RK, W), np.int8)
        _FAST["padT"] = np.zeros((B, H + 2 * RK, W), np.int8)
        _FAST["gP"] = np.empty((N_CORES * ROWS, W), np.int8)
        _FAST["gT"] = np.empty((N_CORES * ROWS, W), np.int8)
    padP, padT = _FAST["padP"], _FAST["padT"]
    gP, gT = _FAST["gP"], _FAST["gT"]
    padP[:, RK : RK + H] = pm
    padT[:, RK : RK + H] = tg
    for core in range(N_CORES):
        b, half = divmod(core, 2)
        r0 = half * 128
        gP[core * ROWS : (core + 1) * ROWS] = padP[b, r0 : r0 + ROWS]
        gT[core * ROWS : (core + 1) * ROWS] = padT[b, r0 : r0 + ROWS]

    outs = _FAST["runner"]([gP, gT])
    return np.float32(float(outs[0].sum()) / (B * H * W))


# ====================================================== general (slow) path

def _row_dists(binary):
    """Per-pixel distance to nearest set pixel in its row (INF if row empty).

    binary: [..., n] bool. Vectorized two-scan min-plus.
    """
    n = binary.shape[-1]
    idx = np.arange(n, dtype=np.int64)
    d = np.where(binary, 0, INF).astype(np.int64)
    fwd = np.minimum.accumulate(d - idx, axis=-1) + idx
    bwd = (
        np.minimum.accumulate((d + idx)[..., ::-1], axis=-1)[..., ::-1] - idx
    )
    return np.minimum(fwd, bwd)


def _plan(pred, target):
    """Choose window radius R and per-(image, mask, class) presence flags."""
    pm = np.argmax(pred, axis=1)
    flags = np.zeros((B, 6), np.float32)
    R = 1
    for mi, mask in enumerate((pm, target)):
        for c in range(1, C):
            slab = mi * 3 + (c - 1)
            b = mask == c
            present = b.any(axis=(1, 2))  # [B]
            flags[:, slab] = present.astype(np.float32)
            if not present.any():
                continue
            dr = _row_dists(b)
            finite = dr < INF // 2
            r1 = int(dr[finite].max()) if finite.any() else 0
            rows_any = b.any(axis=2)  # [B, H]
            vg = 0
            for bi in range(B):
                if not present[bi]:
                    continue
                if not rows_any[bi].all():
                    vg = max(vg, int(_row_dists(rows_any[bi][None])[0].max()))
            R = max(R, min(r1 + vg, 361))
    return R, flags


def _build(R, use_i16, iters=1):
    rows_in = ((128 + 2 * R + 127) // 128) * 128
    capv = 127.0 if use_i16 else 400.0
    padv = 30000 if use_i16 else 1.0e9
    DT = I16 if use_i16 else F32

    nc = bacc.Bacc(None, target_bir_lowering=False)
    predS = nc.dram_tensor("predS", [rows_in, C, W], F32, kind="ExternalInput")
    targS = nc.dram_tensor("targS", [rows_in, W], I32, kind="ExternalInput")
    flagsI = nc.dram_tensor("flags", [128, 6], F32, kind="ExternalInput")
    out = nc.dram_tensor("out", [128, 1], F32, kind="ExternalOutput")

    chunks = list(range(0, rows_in, 128))
    rows_pad = rows_in

    with TileContext(nc) as tc:
        with (
            tc.tile_pool(name="const", bufs=1) as constp,
            tc.tile_pool(name="io", bufs=2) as iop,
            tc.tile_pool(name="p1", bufs=2) as p1p,
            tc.tile_pool(name="h2", bufs=1) as h2p,
            tc.tile_pool(name="fin", bufs=1) as finp,
        ):
            def _body():
                flagst = constp.tile([128, 6], F32)
                nc.gpsimd.dma_start(flagst[:], flagsI[:])
                ones = constp.tile([128, W], F32)
                nc.vector.memset(ones[:], 1.0)

                h2d = [h2p.tile([128, 6, rows_pad], I16, name=f"h2d{w}") for w in range(2)]
                h2A = [h2p.tile([128, 6, rows_pad], DT, name=f"h2A{w}") for w in range(2)]
                h2B = [h2p.tile([128, 6, rows_pad], DT, name=f"h2B{w}") for w in range(2)]
                accs = [h2p.tile([128, 6, 128], DT, name=f"acc{w}") for w in range(2)]
                for wc in range(2):
                    nc.vector.memset(h2B[wc][:], padv)
                    nc.vector.memset(accs[wc][:], padv)

                for cs in chunks:
                    predt = iop.tile([128, C, W], F32, name="predt")
                    nc.gpsimd.dma_start(predt[:], predS[cs : cs + 128])
                    targt = iop.tile([128, W], I32, name="targt")
                    nc.gpsimd.dma_start(targt[:], targS[cs : cs + 128])
                    targf = p1p.tile([128, W], F32, name="targf")
                    nc.scalar.activation(targf[:], targt[:], Act.Copy)

                    t0 = p1p.tile([128, W], F32, name="t0")
                    mx = p1p.tile([128, W], F32, name="mx")
                    nc.vector.tensor_max(t0[:], predt[:, 0], predt[:, 1])
                    nc.vector.tensor_max(mx[:], predt[:, 2], predt[:, 3])
                    nc.vector.tensor_max(mx[:], t0[:], mx[:])

                    for slab in range(6):
                        mi, c = divmod(slab, 3)
                        c += 1
                        f = p1p.tile([128, W], F32, name="fseed")
                        if mi == 1:
                            nc.vector.tensor_scalar(
                                f[:], targf[:], float(c), LARGEF,
                                op0=Alu.not_equal, op1=Alu.mult)
                        else:
                            nc.vector.tensor_tensor(
                                f[:], predt[:, c], mx[:], op=Alu.is_lt)
                            nc.vector.tensor_scalar_mul(f[:], f[:], LARGEF)
                        a = p1p.tile([128, W], F32, name="a")
                        nc.vector.tensor_tensor_scan(
                            a[:], ones[:], f[:], LARGEF,
                            op0=Alu.add, op1=Alu.min)
                        dd = p1p.tile([128, W], F32, name="dd")
                        nc.vector.tensor_tensor_scan(
                            dd[:, ::-1], ones[:], a[:, ::-1], LARGEF,
                            op0=Alu.add, op1=Alu.min)
                        nc.vector.tensor_scalar_min(dd[:], dd[:], capv)
                        ddi = p1p.tile([128, W], I16, name="ddi")
                        nc.gpsimd.tensor_copy(ddi[:], dd[:])

                        for wc in range(2):
                            nc.sync.dma_start_transpose(
                                h2d[wc][:, slab, cs : cs + 128],
                                ddi[:, wc * 128 : (wc + 1) * 128])

                for wc in range(2):
                    nc.scalar.activation(h2A[wc][:], h2d[wc][:], Act.Square)
                    nc.scalar.activation(
                        h2B[wc][:, :, 0 : rows_pad - 1],
                        h2d[wc][:, :, 1:rows_pad], Act.Square)

                ks = [0]
                for k in range(1, R + 1):
                    ks += [k, -k]
                for k in ks:
                    base = R + k
                    kk = k * k
                    for wc in range(2):
                        if use_i16 and base % 2 == 1:
                            src, b0 = h2B[wc], base - 1
                        else:
                            src, b0 = h2A[wc], base
                        nc.vector.scalar_tensor_tensor(
                            accs[wc][:], src[:, :, b0 : b0 + 128],
                            float(kk) if not use_i16 else int(kk),
                            accs[wc][:],
                            op0=Alu.add, op1=Alu.min)

                prt = finp.tile([128, 2], F32)
                for wc in range(2):
                    sq = finp.tile([128, 6, 128], F32, name="sq")
                    for slab in range(6):
                        nc.scalar.activation(
                            sq[:, slab], accs[wc][:, slab], Act.Sqrt)
                        nc.vector.tensor_single_scalar(
                            sq[:, slab], sq[:, slab],
                            flagst[:, slab : slab + 1], op=Alu.mult)
                    sp = finp.tile([128, 128], F32, name="sp")
                    st = finp.tile([128, 128], F32, name="st")
                    nc.vector.tensor_add(sp[:], sq[:, 0], sq[:, 1])
                    nc.vector.tensor_add(sp[:], sp[:], sq[:, 2])
                    nc.vector.tensor_add(st[:], sq[:, 3], sq[:, 4])
                    nc.vector.tensor_add(st[:], st[:], sq[:, 5])
                    nc.vector.tensor_sub(sp[:], sp[:], st[:])
                    nc.vector.tensor_reduce(
                        prt[:, wc : wc + 1], sp[:], axis=mybir.AxisListType.X,
                        op=Alu.add, apply_absolute_value=True)
                total = finp.tile([128, 1], F32)
                nc.vector.tensor_add(total[:], prt[:, 0:1], prt[:, 1:2])
                nc.gpsimd.dma_start(out[:], total[:])

            if iters > 1:
                E = mybir.EngineType
                with tc.For_i(0, iters, 1, hint_engines=(
                        E.DVE, E.Activation, E.Pool, E.SP)):
                    _body()
            else:
                _body()

    nc.finalize()
    return nc, rows_in


_CACHE = {}


def _get_nc(R, use_i16, iters=1):
    key = (R, use_i16, iters)
    if key not in _CACHE:
        _CACHE[key] = _build(R, use_i16, iters)
    return _CACHE[key]


def _make_in_maps(pred, target, flags, R, rows_in):
    in_maps = []
    for core in range(N_CORES):
        b, half = divmod(core, 2)
        r0 = half * 128
        lo, hi = r0 - R, r0 + 128 + R
        clo, chi = max(0, lo), min(H, hi)
        plo = max(0, -lo)
        phi = rows_in - plo - (chi - clo)  # bottom pad up to rows_in
        predS = np.transpose(pred[b, :, clo:chi, :], (1, 0, 2)).astype(
            np.float32, copy=True)
        # pad rows: channel 0 wins -> classes 1..3 seed LARGE
        padrow = np.zeros((1, C, W), np.float32)
        padrow[0, 0, :] = 1.0
        predS = np.concatenate(
            [np.repeat(padrow, plo, 0), predS, np.repeat(padrow, phi, 0)], 0)
        targS = np.pad(
            target[b, clo:chi, :], ((plo, phi), (0, 0)),
            constant_values=-1).astype(np.int32)
        assert predS.shape == (rows_in, C, W) and targS.shape == (rows_in, W)
        fl = np.repeat(flags[b][None, :], 128, 0).astype(np.float32)
        in_maps.append({"predS": predS, "targS": targS, "flags": fl})
    return in_maps


def _slow_call(pred, target):
    R, flags = _plan(pred, target)
    use_i16 = R <= 120
    nc, rows_in = _get_nc(R, use_i16)
    in_maps = _make_in_maps(pred, target, flags, R, rows_in)
    res = run_bass_kernel_spmd(nc, in_maps, list(range(N_CORES)))
    total = sum(float(r["out"].sum()) for r in res.results)
    return np.float32(total / (B * H * W))


def kernel(pred, target):
    pred = np.ascontiguousarray(pred, dtype=np.float32)
    target = np.ascontiguousarray(target, dtype=np.int32)
    out = _fast_call(pred, target)
    if out is None:
        out = _slow_call(pred, target)
    return out


# revision 9
# speedup vs baseline: 1.5086x; 1.5086x over previous
"""BoundaryLoss kernel for 8 Trainium2 NeuronCores.

Computes mean |pred_dist - target_dist| where *_dist are sums of per-class
exact Euclidean distance transforms of the argmax(pred) / target masks.

Sharding: 8 cores = 4 images x 2 H-halves. Each core computes both masks'
3 per-class EDTs for its half (with +-RK halo rows) and reduces to a
[128,1] partial |diff| sum; the host sums 8 partials and divides.

EDT algorithm per (mask, class, image):
  pass 1 (along W): exact nearest-set-pixel row distances via two
    min-plus scans  state = min(state+1, f)  (forward + backward).
  pass 2 (along H): d^2(x) = min_k (dr[x+k]^2 + k^2) windowed to |k| <= R.
    One fused scalar_tensor_tensor per offset k.

Fast path (the common, dense-mask case): host computes argmax class-id
masks (int8) and two cheap coverage checks that make a fixed window
R=64 and an int16 pipeline provably exact:
  - every 64-wide row segment of every class mask contains a set pixel
    (=> all row distances <= 63, no capping),
  - every 16x16 block contains a set pixel (=> true EDT <= 21.3, so the
    nearest set pixel is within 21 rows; window 64 is exact; every class
    present in every image so no presence flags needed).
Inputs shipped per core are just two [256,256] int8 masks (~1MB total
across cores instead of ~10MB of f32 logits), and the jit dispatch
closure is built once and cached (the generic run_bass_kernel_spmd
rebuilds + retraces a fresh jit every call, ~100ms/call overhead).

If the checks fail (sparse/absent classes), falls back to the general
exact path (data-derived R, on-device argmax, presence flags) via
run_bass_kernel_spmd.
"""

import numpy as np

import concourse.bass as bass
import concourse.bacc as bacc
import concourse.mybir as mybir
from concourse.tile import TileContext
from concourse.bass_utils import run_bass_kernel_spmd

B, C, H, W = 4, 4, 256, 256
N_CORES = 8
LARGEF = 1.0e6  # pseudo-infinity seed for pass-1 scans (pre-square space)
INF = 1 << 20

RK = 64              # fast-path fixed pass-2 window radius
ROWS = 128 + 2 * RK  # rows per core incl. halo (= 256)

F32 = mybir.dt.float32
I32 = mybir.dt.int32
I16 = mybir.dt.int16
I8 = mybir.dt.int8
U8 = mybir.dt.uint8
Alu = mybir.AluOpType
Act = mybir.ActivationFunctionType


# ================================================================ fast path

def _build_fast():
    """Fixed-R (=RK) int16 EDT kernel taking 2-bit-packed class-id masks.

    Packed layout: byte x of a row holds pixels x, 64+x, 128+x, 192+x
    (2 bits each, LSB first). Besides the loss partial, outputs the max
    computed distance: if max <= RK the fixed window + int16 cap are
    provably exact for this input (the true nearest pixel is within RK
    rows and no capped entry can win a min below 127), so the host can
    certify the fast result after the fact instead of pre-checking.
    """
    capv = 127.0
    padv = 30000
    rows_pad = ROWS

    nc = bacc.Bacc(None, target_bir_lowering=False)
    maskP = nc.dram_tensor("maskP", [ROWS, W // 4], U8, kind="ExternalInput")
    maskT = nc.dram_tensor("maskT", [ROWS, W // 4], U8, kind="ExternalInput")
    out = nc.dram_tensor("out", [128, 2], F32, kind="ExternalOutput")

    with TileContext(nc) as tc:
        with (
            tc.tile_pool(name="const", bufs=1) as constp,
            tc.tile_pool(name="io", bufs=2) as iop,
            tc.tile_pool(name="p1", bufs=2) as p1p,
            tc.tile_pool(name="h2", bufs=1) as h2p,
            tc.tile_pool(name="fin", bufs=1) as finp,
        ):
            ones = constp.tile([128, W], F32)
            nc.vector.memset(ones[:], 1.0)

            # per-W-chunk transposed row-distance maps, 6 slabs =
            # (pred c1..c3, targ c1..c3). h2A = squared distances; h2B =
            # h2A shifted one element left (keeps odd window offsets on
            # the 2x_1P int16 DVE mode).
            h2d = [h2p.tile([128, 6, rows_pad], I16, name=f"h2d{w}") for w in range(2)]
            h2A = [h2p.tile([128, 6, rows_pad], I16, name=f"h2A{w}") for w in range(2)]
            h2B = [h2p.tile([128, 6, rows_pad], I16, name=f"h2B{w}") for w in range(2)]
            accs = [h2p.tile([128, 6, 128], I16, name=f"acc{w}") for w in range(2)]
            for wc in range(2):
                nc.vector.memset(h2B[wc][:], padv)
                nc.vector.memset(accs[wc][:], padv)

            # ---------------- pass 1 + transpose, per row-chunk
            for cs in (0, 128):
                mpt = iop.tile([128, W // 4], U8, name="mpt")
                nc.gpsimd.dma_start(mpt[:], maskP[cs : cs + 128])
                mtt = iop.tile([128, W // 4], U8, name="mtt")
                nc.gpsimd.dma_start(mtt[:], maskT[cs : cs + 128])
                mfs = []
                for pkt, nm in ((mpt, "p"), (mtt, "t")):
                    pk16 = p1p.tile([128, W // 4], I16, name=f"pk16{nm}")
                    nc.gpsimd.tensor_copy(pk16[:], pkt[:])
                    mcls = p1p.tile([128, W], I16, name=f"mcls{nm}")
                    nc.vector.tensor_scalar(
                        mcls[:, 0:64], pk16[:], 3, None, op0=Alu.bitwise_and)
                    for j in range(1, 4):
                        nc.vector.tensor_scalar(
                            mcls[:, j * 64 : (j + 1) * 64], pk16[:],
                            2 * j, 3,
                            op0=Alu.logical_shift_right, op1=Alu.bitwise_and)
                    mf = p1p.tile([128, W], F32, name=f"mf{nm}")
                    nc.scalar.activation(mf[:], mcls[:], Act.Copy)
                    mfs.append(mf)
                mpf, mtf = mfs

                for slab in range(6):
                    mi, c = divmod(slab, 3)
                    c += 1
                    srcf = mtf if mi == 1 else mpf
                    f = p1p.tile([128, W], F32, name="fseed")
                    nc.vector.tensor_scalar(
                        f[:], srcf[:], float(c), LARGEF,
                        op0=Alu.not_equal, op1=Alu.mult)
                    a = p1p.tile([128, W], F32, name="a")
                    nc.vector.tensor_tensor_scan(
                        a[:], ones[:], f[:], LARGEF,
                        op0=Alu.add, op1=Alu.min)
                    dd = p1p.tile([128, W], F32, name="dd")
                    nc.vector.tensor_tensor_scan(
                        dd[:, ::-1], ones[:], a[:, ::-1], LARGEF,
                        op0=Alu.add, op1=Alu.min)
                    nc.vector.tensor_scalar_min(dd[:], dd[:], capv)
                    ddi = p1p.tile([128, W], I16, name="ddi")
                    nc.gpsimd.tensor_copy(ddi[:], dd[:])

                    for wc in range(2):
                        nc.sync.dma_start_transpose(
                            h2d[wc][:, slab, cs : cs + 128],
                            ddi[:, wc * 128 : (wc + 1) * 128])

            # squares: h2A = h2d^2, h2B = shifted h2A
            for wc in range(2):
                nc.scalar.activation(h2A[wc][:], h2d[wc][:], Act.Square)
                nc.scalar.activation(
                    h2B[wc][:, :, 0 : rows_pad - 1],
                    h2d[wc][:, :, 1:rows_pad], Act.Square)

            # ---------------- pass 2: windowed parabola min-plus along H
            ks = [0]
            for k in range(1, RK + 1):
                ks += [k, -k]
            for k in ks:
                base = RK + k
                kk = k * k
                for wc in range(2):
                    if base % 2 == 1:
                        src, b0 = h2B[wc], base - 1
                    else:
                        src, b0 = h2A[wc], base
                    nc.vector.scalar_tensor_tensor(
                        accs[wc][:], src[:, :, b0 : b0 + 128],
                        int(kk), accs[wc][:],
                        op0=Alu.add, op1=Alu.min)

            # ---------------- sqrt, class sums, |pred-targ|, reduce
            prt = finp.tile([128, 2], F32)
            mxp = finp.tile([128, 2], F32)
            for wc in range(2):
                sq = finp.tile([128, 6, 128], F32, name="sq")
                for slab in range(6):
                    nc.scalar.activation(
                        sq[:, slab], accs[wc][:, slab], Act.Sqrt)
                sp = finp.tile([128, 128], F32, name="sp")
                st = finp.tile([128, 128], F32, name="st")
                mxt = finp.tile([128, 128], F32, name="mxt")
                nc.vector.tensor_max(mxt[:], sq[:, 0], sq[:, 1])
                nc.vector.tensor_max(mxt[:], mxt[:], sq[:, 2])
                nc.vector.tensor_max(mxt[:], mxt[:], sq[:, 3])
                nc.vector.tensor_max(mxt[:], mxt[:], sq[:, 4])
                nc.vector.tensor_max(mxt[:], mxt[:], sq[:, 5])
                nc.vector.tensor_reduce(
                    mxp[:, wc : wc + 1], mxt[:], axis=mybir.AxisListType.X,
                    op=Alu.max)
                nc.vector.tensor_add(sp[:], sq[:, 0], sq[:, 1])
                nc.vector.tensor_add(sp[:], sp[:], sq[:, 2])
                nc.vector.tensor_add(st[:], sq[:, 3], sq[:, 4])
                nc.vector.tensor_add(st[:], st[:], sq[:, 5])
                nc.vector.tensor_sub(sp[:], sp[:], st[:])
                nc.vector.tensor_reduce(
                    prt[:, wc : wc + 1], sp[:], axis=mybir.AxisListType.X,
                    op=Alu.add, apply_absolute_value=True)
            total = finp.tile([128, 2], F32)
            nc.vector.tensor_add(total[:, 0:1], prt[:, 0:1], prt[:, 1:2])
            nc.vector.tensor_max(total[:, 1:2], mxp[:, 0:1], mxp[:, 1:2])
            nc.gpsimd.dma_start(out[:], total[:])

    nc.finalize()
    return nc


def _build_runner(nc, n_cores):
    """Build the PJRT dispatch closure ONCE (same lowering path as
    bass_utils.run_bass_kernel_spmd -> bass2jax.run_bass_via_pjrt, with
    the jit hoisted out of the per-call path)."""
    import jax
    from jax.sharding import Mesh, PartitionSpec
    from jax.experimental.shard_map import shard_map
    from concourse import bass2jax as b2j

    b2j.install_neuronx_cc_hook()
    assert nc.dbg_addr is None
    partition_name = (
        nc.partition_id_tensor.name if nc.partition_id_tensor else None
    )

    in_names, out_names, out_avals = [], [], []
    for alloc in nc.m.functions[0].allocations:
        if not isinstance(alloc, mybir.MemoryLocationSet):
            continue
        name = alloc.memorylocations[0].name
        if alloc.kind == "ExternalInput":
            if name != partition_name:
                in_names.append(name)
        elif alloc.kind == "ExternalOutput":
            out_names.append(name)
            shape = tuple(alloc.tensor_shape)
            dtype = mybir.dt.np(alloc.dtype)
            out_avals.append(jax.core.ShapedArray(shape, dtype))
    n_params = len(in_names)
    n_outs = len(out_avals)
    in_names_all = list(in_names) + list(out_names)
    if partition_name is not None:
        in_names_all.append(partition_name)
    in_names_all = tuple(in_names_all)
    donate = tuple(range(n_params, n_params + n_outs))

    def _body(*args):
        operands = list(args)
        if partition_name is not None:
            operands.append(b2j.partition_id_tensor())
        outs = b2j._bass_exec_p.bind(
            *operands,
            out_avals=tuple(out_avals),
            in_names=in_names_all,
            out_names=tuple(out_names),
            lowering_input_output_aliases=(),
            sim_require_finite=True,
            sim_require_nnan=True,
            nc=nc,
        )
        return tuple(outs)

    devices = jax.devices()[:n_cores]
    mesh = Mesh(np.asarray(devices), ("core",))
    sharded = jax.jit(
        shard_map(
            _body, mesh=mesh,
            in_specs=(PartitionSpec("core"),) * (n_params + n_outs),
            out_specs=(PartitionSpec("core"),) * n_outs,
            check_rep=False,
        ),
        donate_argnums=donate, keep_unused=True,
    )
    zero_shapes = [
        ((n_cores * a.shape[0], *a.shape[1:]), a.dtype) for a in out_avals
    ]

    def run(global_inputs):
        """global_inputs: list of [n_cores*dim0, ...] arrays in in_names
        order. Returns list of global output arrays (host numpy)."""
        zeros = [np.zeros(s, d) for s, d in zero_shapes]
        out_arrs = sharded(*global_inputs, *zeros)
        return [np.asarray(o) for o in out_arrs]

    return run


_FAST = {}


def _fast_call(pred, target):
    """Returns the loss, or None if this input needs the general path."""
    if not _FAST:
        shp = (B, H, W)
        for k in ("m01", "m23"):
            _FAST[k] = np.empty(shp, np.float32)
        for k in ("hi", "i01", "i23", "low"):
            _FAST[k] = np.empty(shp, bool)
        _FAST["pm8"] = np.empty(shp, np.uint8)
        _FAST["tg8"] = np.empty(shp, np.uint8)
        _FAST["padP"] = np.zeros((B, H + 2 * RK, W // 4), np.uint8)
        _FAST["padT"] = np.zeros((B, H + 2 * RK, W // 4), np.uint8)
        _FAST["gP"] = np.empty((N_CORES * ROWS, W // 4), np.uint8)
        _FAST["gT"] = np.empty((N_CORES * ROWS, W // 4), np.uint8)
        _FAST["runner"] = _build_runner(_build_fast(), N_CORES)
    f = _FAST

    # argmax over the 4 classes (first-wins ties, matches np.argmax)
    p0, p1 = pred[:, 0], pred[:, 1]
    p2, p3 = pred[:, 2], pred[:, 3]
    m01, m23 = f["m01"], f["m23"]
    hi, i01, i23, low = f["hi"], f["i01"], f["i23"], f["low"]
    pm8, tg8 = f["pm8"], f["tg8"]
    np.maximum(p0, p1, out=m01)
    np.maximum(p2, p3, out=m23)
    np.greater(m23, m01, out=hi)
    np.greater(p1, p0, out=i01)
    np.greater(p3, p2, out=i23)
    np.copyto(low, i01)
    np.copyto(low, i23, where=hi)
    np.left_shift(hi.view(np.uint8), 1, out=pm8)
    np.add(pm8, low.view(np.uint8), out=pm8)
    np.copyto(tg8, target, casting="unsafe")

    # pack 4 pixels/byte (strided: byte x holds pixels x, 64+x, 128+x, 192+x)
    padP, padT = f["padP"], f["padT"]
    for src, dst in ((pm8, padP), (tg8, padT)):
        s4 = src.reshape(B, H, 4, 64)
        pk = dst[:, RK : RK + H]
        np.left_shift(s4[:, :, 3], 6, out=pk)
        np.bitwise_or(pk, s4[:, :, 2] << 4, out=pk)
        np.bitwise_or(pk, s4[:, :, 1] << 2, out=pk)
        np.bitwise_or(pk, s4[:, :, 0], out=pk)
    gP, gT = f["gP"], f["gT"]
    for core in range(N_CORES):
        b, half = divmod(core, 2)
        r0 = half * 128
        gP[core * ROWS : (core + 1) * ROWS] = padP[b, r0 : r0 + ROWS]
        gT[core * ROWS : (core + 1) * ROWS] = padT[b, r0 : r0 + ROWS]

    o = _FAST["runner"]([gP, gT])[0]  # [8*128, 2]: (sum partial, max dist)
    if not (o[:, 1].max() <= float(RK)):
        return None  # window/cap not certified exact for this input
    return np.float32(float(o[:, 0].sum()) / (B * H * W))


# ====================================================== general (slow) path

def _row_dists(binary):
    """Per-pixel distance to nearest set pixel in its row (INF if row empty).

    binary: [..., n] bool. Vectorized two-scan min-plus.
    """
    n = binary.shape[-1]
    idx = np.arange(n, dtype=np.int64)
    d = np.where(binary, 0, INF).astype(np.int64)
    fwd = np.minimum.accumulate(d - idx, axis=-1) + idx
    bwd = (
        np.minimum.accumulate((d + idx)[..., ::-1], axis=-1)[..., ::-1] - idx
    )
    return np.minimum(fwd, bwd)


def _plan(pred, target):
    """Choose window radius R and per-(image, mask, class) presence flags."""
    pm = np.argmax(pred, axis=1)
    flags = np.zeros((B, 6), np.float32)
    R = 1
    for mi, mask in enumerate((pm, target)):
        for c in range(1, C):
            slab = mi * 3 + (c - 1)
            b = mask == c
            present = b.any(axis=(1, 2))  # [B]
            flags[:, slab] = present.astype(np.float32)
            if not present.any():
                continue
            dr = _row_dists(b)
            finite = dr < INF // 2
            r1 = int(dr[finite].max()) if finite.any() else 0
            rows_any = b.any(axis=2)  # [B, H]
            vg = 0
            for bi in range(B):
                if not present[bi]:
                    continue
                if not rows_any[bi].all():
                    vg = max(vg, int(_row_dists(rows_any[bi][None])[0].max()))
            R = max(R, min(r1 + vg, 361))
    return R, flags


def _build(R, use_i16, iters=1):
    rows_in = ((128 + 2 * R + 127) // 128) * 128
    capv = 127.0 if use_i16 else 400.0
    padv = 30000 if use_i16 else 1.0e9
    DT = I16 if use_i16 else F32

    nc = bacc.Bacc(None, target_bir_lowering=False)
    predS = nc.dram_tensor("predS", [rows_in, C, W], F32, kind="ExternalInput")
    targS = nc.dram_tensor("targS", [rows_in, W], I32, kind="ExternalInput")
    flagsI = nc.dram_tensor("flags", [128, 6], F32, kind="ExternalInput")
    out = nc.dram_tensor("out", [128, 1], F32, kind="ExternalOutput")

    chunks = list(range(0, rows_in, 128))
    rows_pad = rows_in

    with TileContext(nc) as tc:
        with (
            tc.tile_pool(name="const", bufs=1) as constp,
            tc.tile_pool(name="io", bufs=2) as iop,
            tc.tile_pool(name="p1", bufs=2) as p1p,
            tc.tile_pool(name="h2", bufs=1) as h2p,
            tc.tile_pool(name="fin", bufs=1) as finp,
        ):
            def _body():
                flagst = constp.tile([128, 6], F32)
                nc.gpsimd.dma_start(flagst[:], flagsI[:])
                ones = constp.tile([128, W], F32)
                nc.vector.memset(ones[:], 1.0)

                h2d = [h2p.tile([128, 6, rows_pad], I16, name=f"h2d{w}") for w in range(2)]
                h2A = [h2p.tile([128, 6, rows_pad], DT, name=f"h2A{w}") for w in range(2)]
                h2B = [h2p.tile([128, 6, rows_pad], DT, name=f"h2B{w}") for w in range(2)]
                accs = [h2p.tile([128, 6, 128], DT, name=f"acc{w}") for w in range(2)]
                for wc in range(2):
                    nc.vector.memset(h2B[wc][:], padv)
                    nc.vector.memset(accs[wc][:], padv)

                for cs in chunks:
                    predt = iop.tile([128, C, W], F32, name="predt")
                    nc.gpsimd.dma_start(predt[:], predS[cs : cs + 128])
                    targt = iop.tile([128, W], I32, name="targt")
                    nc.gpsimd.dma_start(targt[:], targS[cs : cs + 128])
                    targf = p1p.tile([128, W], F32, name="targf")
                    nc.scalar.activation(targf[:], targt[:], Act.Copy)

                    t0 = p1p.tile([128, W], F32, name="t0")
                    mx = p1p.tile([128, W], F32, name="mx")
                    nc.vector.tensor_max(t0[:], predt[:, 0], predt[:, 1])
                    nc.vector.tensor_max(mx[:], predt[:, 2], predt[:, 3])
                    nc.vector.tensor_max(mx[:], t0[:], mx[:])

                    for slab in range(6):
                        mi, c = divmod(slab, 3)
                        c += 1
                        f = p1p.tile([128, W], F32, name="fseed")
                        if mi == 1:
                            nc.vector.tensor_scalar(
                                f[:], targf[:], float(c), LARGEF,
                                op0=Alu.not_equal, op1=Alu.mult)
                        else:
                            nc.vector.tensor_tensor(
                                f[:], predt[:, c], mx[:], op=Alu.is_lt)
                            nc.vector.tensor_scalar_mul(f[:], f[:], LARGEF)
                        a = p1p.tile([128, W], F32, name="a")
                        nc.vector.tensor_tensor_scan(
                            a[:], ones[:], f[:], LARGEF,
                            op0=Alu.add, op1=Alu.min)
                        dd = p1p.tile([128, W], F32, name="dd")
                        nc.vector.tensor_tensor_scan(
                            dd[:, ::-1], ones[:], a[:, ::-1], LARGEF,
                            op0=Alu.add, op1=Alu.min)
                        nc.vector.tensor_scalar_min(dd[:], dd[:], capv)
                        ddi = p1p.tile([128, W], I16, name="ddi")
                        nc.gpsimd.tensor_copy(ddi[:], dd[:])

                        for wc in range(2):
                            nc.sync.dma_start_transpose(
                                h2d[wc][:, slab, cs : cs + 128],
                                ddi[:, wc * 128 : (wc + 1) * 128])

                for wc in range(2):
                    nc.scalar.activation(h2A[wc][:], h2d[wc][:], Act.Square)
                    nc.scalar.activation(
                        h2B[wc][:, :, 0 : rows_pad - 1],
                        h2d[wc][:, :, 1:rows_pad], Act.Square)

                ks = [0]
                for k in range(1, R + 1):
                    ks += [k, -k]
                for k in ks:
                    base = R + k
                    kk = k * k
                    for wc in range(2):
                        if use_i16 and base % 2 == 1:
                            src, b0 = h2B[wc], base - 1
                        else:
                            src, b0 = h2A[wc], base
                        nc.vector.scalar_tensor_tensor(
                            accs[wc][:], src[:, :, b0 : b0 + 128],
                            float(kk) if not use_i16 else int(kk),
                            accs[wc][:],
                            op0=Alu.add, op1=Alu.min)

                prt = finp.tile([128, 2], F32)
                for wc in range(2):
                    sq = finp.tile([128, 6, 128], F32, name="sq")
                    for slab in range(6):
                        nc.scalar.activation(
                            sq[:, slab], accs[wc][:, slab], Act.Sqrt)
                        nc.vector.tensor_single_scalar(
                            sq[:, slab], sq[:, slab],
                            flagst[:, slab : slab + 1], op=Alu.mult)
                    sp = finp.tile([128, 128], F32, name="sp")
                    st = finp.tile([128, 128], F32, name="st")
                    nc.vector.tensor_add(sp[:], sq[:, 0], sq[:, 1])
                    nc.vector.tensor_add(sp[:], sp[:], sq[:, 2])
                    nc.vector.tensor_add(st[:], sq[:, 3], sq[:, 4])
                    nc.vector.tensor_add(st[:], st[:], sq[:, 5])
                    nc.vector.tensor_sub(sp[:], sp[:], st[:])
                    nc.vector.tensor_reduce(
                        prt[:, wc : wc + 1], sp[:], axis=mybir.AxisListType.X,
                        op=Alu.add, apply_absolute_value=True)
                total = finp.tile([128, 1], F32)
                nc.vector.tensor_add(total[:], prt[:, 0:1], prt[:, 1:2])
                nc.gpsimd.dma_start(out[:], total[:])

            if iters > 1:
                E = mybir.EngineType
                with tc.For_i(0, iters, 1, hint_engines=(
                        E.DVE, E.Activation, E.Pool, E.SP)):
                    _body()
            else:
                _body()

    nc.finalize()
    return nc, rows_in


_CACHE = {}


def _get_nc(R, use_i16, iters=1):
    key = (R, use_i16, iters)
    if key not in _CACHE:
        _CACHE[key] = _build(R, use_i16, iters)
    return _CACHE[key]


def _make_in_maps(pred, target, flags, R, rows_in):
    in_maps = []
    for core in range(N_CORES):
        b, half = divmod(core, 2)
        r0 = half * 128
        lo, hi = r0 - R, r0 + 128 + R
        clo, chi = max(0, lo), min(H, hi)
        plo = max(0, -lo)
        phi = rows_in - plo - (chi - clo)  # bottom pad up to rows_in
        predS = np.transpose(pred[b, :, clo:chi, :], (1, 0, 2)).astype(
            np.float32, copy=True)
        # pad rows: channel 0 wins -> classes 1..3 seed LARGE
        padrow = np.zeros((1, C, W), np.float32)
        padrow[0, 0, :] = 1.0
        predS = np.concatenate(
            [np.repeat(padrow, plo, 0), predS, np.repeat(padrow, phi, 0)], 0)
        targS = np.pad(
            target[b, clo:chi, :], ((plo, phi), (0, 0)),
            constant_values=-1).astype(np.int32)
        assert predS.shape == (rows_in, C, W) and targS.shape == (rows_in, W)
        fl = np.repeat(flags[b][None, :], 128, 0).astype(np.float32)
        in_maps.append({"predS": predS, "targS": targS, "flags": fl})
    return in_maps


def _slow_call(pred, target):
    R, flags = _plan(pred, target)
    use_i16 = R <= 120
    nc, rows_in = _get_nc(R, use_i16)
    in_maps = _make_in_maps(pred, target, flags, R, rows_in)
    res = run_bass_kernel_spmd(nc, in_maps, list(range(N_CORES)))
    total = sum(float(r["out"].sum()) for r in res.results)
    return np.float32(total / (B * H * W))


def kernel(pred, target):
    pred = np.ascontiguousarray(pred, dtype=np.float32)
    target = np.ascontiguousarray(target, dtype=np.int32)
    out = _fast_call(pred, target)
    if out is None:
        out = _slow_call(pred, target)
    return out
